# revision 32
# baseline (speedup 1.0000x reference)
"""Trainium2 Bass kernel for nn_GeneralizedKernelScore (loss_fn).

Math per sample n (M=8 population members, D=12288 features):
    beta      = 2.0 - 1.9*t/999                      (linear schedule from t)
    conf[n]   = mean_j    exp(-beta*||x_j - y_j||^2 / D)
    inter[n]  = mean_{j!=j'} exp(-beta*||x_j - x_j'||^2 / D)
    im[n]     = inter/2
    score[n]  = im - conf

Strategy (data-parallel over batch, 4 samples per core on 8 cores):
Each core owns Z = [X; Y] (64 rows x 12288) in fp8-e4m3 (host casts;
quantization keeps worst rel err ~5e-4, well under the 2e-2 gate).
Every distance comes from the 64x64 Gram matrix G = Z Z^T:
    ||z_a - z_b||^2 = G[a,a] + G[b,b] - 2 G[a,b]
G is accumulated as 48 fp8 matmuls over PAIRS of 128-wide feature
chunks: the stationary is [128 feat, 128 cols] covering two chunks,
and the two diagonal 64x64 quadrants of the [128,128] PSUM tile hold
the two chunks' Gram contributions; off-diagonal quadrants are
cross-chunk garbage that is never read.  (One matmul per pair halves
the LDWEIGHTS+dispatch overhead vs per-chunk matmuls: 107ns vs 128ns
per two chunks, measured.)  The quadrants are never folded into one G:
the masked-reduce extraction runs on both quadrants (stacked on
partitions 0:64 / 64:128), and the tiny fp16 selection matmuls that
build the distance args contract over all 128 partitions with a
stacked identity, summing the two quadrants' contributions for free.

Input DMA is 6 non-uniform chunks on the Sync HWDGE ring, sized so
each chunk's arrival (issue + latency + wire + completion semaphore)
lands just before the 107ns/pair Gram consumption needs it; the
constants follow on the same ring so they never compete with input
chunks at the SDMA engines.  One ScalarE exp (scale = -beta/D,
per-partition bias = -beta/D * ||x_p||^2) evaluates all 9 distance
columns; the per-sample matmul + a column reduce finish the means.
"""

from contextlib import ExitStack

import numpy as np

import concourse.mybir as mybir
import concourse.tile as tile
from concourse import bacc
from concourse.bass_utils import run_bass_kernel_spmd

# problem shape (hardcoded per spec)
N, M, D = 32, 8, 12288
NUM_TIMESTEPS = 1000
BETA_START, BETA_END = 2.0, 0.1
LAMBDA_VAL = 1.0

NCORES = 8
NS = N // NCORES          # 4 samples per core
R = 2 * NS * M            # 64 Z-rows per core (32 x-rows then 32 y-rows)
NCH = D // 128            # 96 contraction chunks

FREE = NCH * R            # 6144 free columns of Z^T
NPAIR = NCH // 2          # 48 chunk pairs (one matmul each)
# Non-uniform input DMA chunks (columns of zt): sized so each chunk's
# arrival (issue + ~0.55us DMA latency + wire time + ~0.45us completion
# semaphore) lands just before the Gram matmuls (107ns/pair) need it.
CHUNK_COLS = [1024, 1152, 1280, 1280, 896, 512]
CHUNK_PAIRS = [c // 128 for c in CHUNK_COLS]
assert sum(CHUNK_COLS) == FREE

# sel (f32) column layout: AAh(-0.5) | I4 | I4h(-0.5) | P4 | J8full
_A0, _I0, _H0, _P0, _J0 = 0, 32, 64, 96, 100
SELW = 108
# msk (bf16) column layout: BLK[32] | XY[32] | J8[8]
_B0, _X0, _JM = 0, 32, 64
MSKW = 72

IN_MODE = "fp8"           # "fp8" or "bf16" input/matmul dtype

F32 = mybir.dt.float32
F16 = mybir.dt.float16
BF16 = mybir.dt.bfloat16
FP8 = mybir.dt.float8e4


def _build_consts():
    p = np.arange(128)[:, None]
    i32 = np.arange(32)[None, :]
    blk = np.where(p < 64, p, p - 64)  # row index within quadrant
    inq = (p < 32) | ((p >= 64) & (p < 96))  # x-rows of either quadrant
    # AAh: sample-block selector (x-rows, both quadrants) scaled -0.5 to
    # undo the -2 baked into the block mask (rhsj carries -2*||x||^2)
    AA = (((blk // 8) == (i32 // 8)) & inq) * -0.5
    # I4: identity on each 32-row block -> matmul against a [128,1]
    # column sums the four blocks' entries (quadrant fold for free)
    I4 = ((p % 32) == i32) * 1.0
    I4h = I4 * -0.5
    P4 = ((p // 8) == np.arange(4)[None, :]) & (p < 32)
    J8 = ((blk % 8) == np.arange(8)[None, :]) & inq
    sel = np.concatenate([AA, I4, I4h, P4, J8], axis=1).astype(np.float16)

    # masks per quadrant row layout (rows 0:64 = quadrant 0, stacked):
    # BLK (cols 0:32): x-rows get -2 on their sample's 8 cols (diag
    #   included -> cm2[p, j(p)] = -2||x_p||^2 doubles as the norm source)
    # XY (cols 32:64): x-row p gets -2 at col p (the <x,y> cell of the
    #   quadrant's off-diag block); y-row q gets +1 at col q (||y||^2)
    q = np.arange(64)[:, None]
    BLKq = np.where(
        (q < 32) & ((q // 8) == (np.arange(32)[None, :] // 8)), -2.0, 0.0
    )
    XYq = np.zeros((64, 32), np.float32)
    for pp in range(32):
        XYq[pp, pp] = -2.0
        XYq[32 + pp, pp] = 1.0
    J8q = ((q % 8) == np.arange(8)[None, :]) & (q < 32)
    top = np.concatenate([BLKq, XYq, J8q], axis=1)  # [64, 72]
    msk = np.concatenate([top, top], axis=0)        # [128, 72] both quadrants
    import ml_dtypes

    return sel, msk.astype(ml_dtypes.bfloat16)


def _build_program(in_mode=IN_MODE):
    nc = bacc.Bacc("TRN2", target_bir_lowering=False)
    zdt = FP8 if in_mode == "fp8" else BF16
    zt = nc.dram_tensor("zt", [128, FREE], zdt, kind="ExternalInput")
    tq = nc.dram_tensor("tq", [N, 1], mybir.dt.int32, kind="ExternalInput")
    sel_d = nc.dram_tensor("sel", [128, SELW], F16, kind="ExternalInput")
    msk_d = nc.dram_tensor("msk", [128, MSKW], BF16, kind="ExternalInput")
    res_d = nc.dram_tensor("res", [NS, 4], F32, kind="ExternalOutput")

    add, mult, sub = (
        mybir.AluOpType.add,
        mybir.AluOpType.mult,
        mybir.AluOpType.subtract,
    )
    EXP = mybir.ActivationFunctionType.Exp

    with ExitStack() as ctx:
        tc = ctx.enter_context(tile.TileContext(nc))
        small = ctx.enter_context(tc.tile_pool(name="small", bufs=1))
        zin_p = ctx.enter_context(tc.tile_pool(name="zin", bufs=1))
        psum = ctx.enter_context(tc.tile_pool(name="psum", bufs=1, space="PSUM"))

        # --- input DMA first, on the Sync HWDGE ring ----------------------
        zc = []
        off = 0
        for i, cols in enumerate(CHUNK_COLS):
            z = zin_p.tile([128, cols], zdt, tag=f"zc{i}")
            nc.sync.dma_start(out=z, in_=zt[:, off : off + cols])
            zc.append(z)
            off += cols

        # tq rides the Scalar HWDGE ring (tiny, needed early for beta).
        # sel/msk aren't needed until extraction (~15us): issue them on
        # the Sync ring AFTER all zt chunks (FIFO per ring), so the big
        # const transfers never compete with input chunks at the SDMA
        # engines.
        tq_sb = small.tile([N, 1], mybir.dt.int32, tag="tq")
        nc.scalar.dma_start(out=tq_sb, in_=tq[:])
        sel = small.tile([128, SELW], F16, tag="sel")
        nc.sync.dma_start(out=sel, in_=sel_d[:])
        msk = small.tile([128, MSKW], BF16, tag="msk")
        nc.sync.dma_start(out=msk, in_=msk_d[:])

        # preload the Exp LUT while DMAs run
        warm = small.tile([1, 1], F32, tag="warm")
        nc.vector.memset(warm, 0.0)
        nc.scalar.activation(out=warm, in_=warm, func=EXP)

        npair = float(M * (M - 1))

        # beta pipeline: bvec[p] = -beta[s(p)]/D on partitions 0:32
        tf = small.tile([N, 1], F32, tag="tf")
        nc.vector.tensor_copy(out=tf, in_=tq_sb)  # int32 -> f32
        bvec = small.tile([N, 1], F32, tag="bvec")
        nc.vector.tensor_scalar(
            out=bvec,
            in0=tf,
            scalar1=(BETA_START - BETA_END) / ((NUM_TIMESTEPS - 1) * D),
            scalar2=-BETA_START / D,
            op0=mult,
            op1=add,
        )
        # --- Gram: P[128,128] += S^T S over 48 chunk pairs ----------------
        P = psum.tile([128, 128], F32, tag="P")
        k = 0
        for c, npr in enumerate(CHUNK_PAIRS):
            for j in range(npr):
                sl = zc[c][:, j * 128 : (j + 1) * 128]
                nc.tensor.matmul(
                    P, lhsT=sl, rhs=sl, start=(k == 0), stop=(k == NPAIR - 1)
                )
                k += 1

        # --- extraction on both quadrants (stacked on partitions) ---------
        # One masked multiply per quadrant covers everything: cols 0:32
        # hold the -2*G sample blocks (diag included), cols 32:64 hold
        # -2<x,y> (x-rows) / ||y||^2 (y-rows).
        m1big = small.tile([128, 64], F32, tag="m1big")
        nc.vector.tensor_tensor(
            out=m1big[0:64, :], in0=P[0:64, 0:64],
            in1=msk[0:64, 0:64], op=mult,
        )
        nc.vector.tensor_tensor(
            out=m1big[64:128, :], in0=P[64:128, 64:128],
            in1=msk[64:128, 0:64], op=mult,
        )
        # cm2[p, f] = -2 <x_p, x_{s(p)*8+f}> (per quadrant)
        cm2 = small.tile([128, 8], F16, tag="cm2")
        with nc.allow_low_precision("fp16 holds ~5e4 Gram entries at 5e-4 rel"):
            nc.vector.reduce_sum(
                out=cm2,
                in_=m1big[:, 0:32].rearrange("p (g f) -> p f g", g=NS),
                axis=mybir.AxisListType.X,
            )
        # One STT gives both: its main out cm2*J8 IS rhsj (the -2||x||^2
        # value at col j(p), zero elsewhere), and its accumulator gives
        # xnq[p] = cm2[p, j(p)] = -2 ||x_p||^2 for the exp bias.
        rhsj = small.tile([128, 8], F16, tag="rhsj")
        xnq = small.tile([128, 1], F16, tag="xnq")
        with nc.allow_low_precision("fp16 holds ~2.5e4 norms at 5e-4 rel"):
            nc.vector.scalar_tensor_tensor(
                out=rhsj, in0=cm2, scalar=1.0, in1=msk[:, _JM : _JM + 8],
                op0=mult, op1=mult, accum_out=xnq,
            )
        # r1xy[p] = -2<x,y> (x-rows) / ||y||^2 (y-rows), per quadrant
        r1xy = small.tile([128, 1], F16, tag="r1xy")
        with nc.allow_low_precision("fp16 holds ~2.5e4 norms at 5e-4 rel"):
            nc.vector.reduce_sum(
                out=r1xy, in_=m1big[:, 32:64], axis=mybir.AxisListType.X
            )

        # --- selection matmuls (each also folds the two quadrants) --------
        # xnp[p] = ||x_p||^2 total;  P9[:,8] = ||y_p||^2 - 2<x_p,y_p>;
        # P9[:,0:8] = ||x_{s,f}||^2 - 2<x_p, x_{s,f}>
        xnp = psum.tile([32, 1], F32, tag="xnp")
        nc.tensor.matmul(
            xnp, lhsT=sel[:, _H0 : _H0 + 32], rhs=xnq,
            start=True, stop=True,
        )
        P9 = psum.tile([32, 9], F32, tag="P9")
        nc.tensor.matmul(
            P9[:, 8:9], lhsT=sel[:, _I0 : _I0 + 32], rhs=r1xy,
            start=True, stop=True,
        )
        nc.tensor.matmul(
            P9[:, 0:8], lhsT=sel[:, _I0 : _I0 + 32], rhs=cm2,
            start=True, stop=False,
        )
        nc.tensor.matmul(
            P9[:, 0:8], lhsT=sel[:, _A0 : _A0 + 32], rhs=rhsj,
            start=False, stop=True,
        )

        # bias = -beta/D * ||x_p||^2 folds the per-row norm into the exp
        bxn = small.tile([32, 1], F32, tag="bxn")
        nc.vector.tensor_tensor(out=bxn, in0=bvec, in1=xnp, op=mult)

        # e9 = exp(-beta/D * (d2 terms)); cols 0:8 pair args, col 8 xy arg
        e9 = small.tile([32, 9], F16, tag="e9")
        nc.scalar.activation(
            out=e9, in_=P9, func=EXP, scale=bvec, bias=bxn
        )

        # per-sample sums over the 8 population rows
        psm9 = psum.tile([NS, 9], F32, tag="psm9")
        nc.tensor.matmul(
            psm9, lhsT=sel[0:32, _P0 : _P0 + NS], rhs=e9, start=True, stop=True
        )

        # finals: [score, conf, inter, inter_mult]
        pr = small.tile([NS, 1], F32, tag="pr")
        nc.vector.reduce_sum(
            out=pr, in_=psm9[:, 0:8], axis=mybir.AxisListType.X
        )
        fin = small.tile([NS, 4], F32, tag="fin")
        nc.vector.tensor_scalar(
            out=fin[:, 1:2], in0=psm9[:, 8:9], scalar1=1.0 / M, scalar2=None,
            op0=mult,
        )
        nc.vector.tensor_scalar(
            out=fin[:, 2:3], in0=pr,
            scalar1=1.0 / npair, scalar2=-M / npair, op0=mult, op1=add,
        )
        half_lam = LAMBDA_VAL / 2.0
        nc.vector.tensor_scalar(
            out=fin[:, 3:4], in0=pr,
            scalar1=half_lam / npair, scalar2=-M * half_lam / npair,
            op0=mult, op1=add,
        )
        nc.vector.tensor_tensor(
            out=fin[:, 0:1], in0=fin[:, 3:4], in1=fin[:, 1:2], op=sub
        )
        nc.scalar.dma_start(out=res_d[:], in_=fin)

    nc.compile()
    return nc


_PROG = {}
_CONSTS = None


def _get_prog(in_mode=IN_MODE):
    if in_mode not in _PROG:
        _PROG[in_mode] = _build_program(in_mode)
    return _PROG[in_mode]


def _make_in_maps(x, y, t, in_mode=IN_MODE):
    global _CONSTS
    if _CONSTS is None:
        _CONSTS = _build_consts()
    sel, msk = _CONSTS
    import ml_dtypes

    zdt = ml_dtypes.float8_e4m3 if in_mode == "fp8" else ml_dtypes.bfloat16
    in_maps = []
    for c in range(NCORES):
        xc = x[c * NS : (c + 1) * NS].reshape(NS * M, D)
        yc = y[c * NS : (c + 1) * NS].reshape(NS * M, D)
        z = np.concatenate([xc, yc], axis=0)  # [64, D]
        # feature-major: zt[p, k*64 + r] = z[r, k*128 + p]
        ztc = np.ascontiguousarray(
            z.reshape(R, NCH, 128).transpose(2, 1, 0).reshape(128, FREE),
            dtype=zdt,
        )
        trep = np.repeat(t[c * NS : (c + 1) * NS], M).reshape(N, 1)
        in_maps.append(
            {
                "zt": ztc,
                "tq": np.ascontiguousarray(trep, dtype=np.int32),
                "sel": sel,
                "msk": msk,
            }
        )
    return in_maps


def _run(x, y, t, trace=False, in_mode=IN_MODE, **spmd_kwargs):
    x = np.asarray(x, dtype=np.float32)
    y = np.asarray(y, dtype=np.float32)
    t = np.asarray(t, dtype=np.int32)
    nc = _get_prog(in_mode)
    in_maps = _make_in_maps(x, y, t, in_mode)
    br = run_bass_kernel_spmd(
        nc, in_maps, list(range(NCORES)), trace=trace, **spmd_kwargs
    )
    out = np.concatenate(
        [np.asarray(r["res"], dtype=np.float32) for r in br.results], axis=0
    )  # [32, 4]
    outs = tuple(np.ascontiguousarray(out[:, i]) for i in range(4))
    return outs, br


def kernel(x, y, t):
    """(score, confinement, interaction, interaction_mult), each [32] f32."""
    outs, _ = _run(x, y, t)
    return outs


# revision 33
# speedup vs baseline: 1.1845x; 1.1845x over previous
"""Trainium2 Bass kernel for nn_GeneralizedKernelScore (loss_fn).

Math per sample n (M=8 population members, D=12288 features):
    beta      = 2.0 - 1.9*t/999                      (linear schedule from t)
    conf[n]   = mean_j    exp(-beta*||x_j - y_j||^2 / D)
    inter[n]  = mean_{j!=j'} exp(-beta*||x_j - x_j'||^2 / D)
    im[n]     = inter/2
    score[n]  = im - conf

Strategy (data-parallel over batch, 4 samples per core on 8 cores):
Each core owns Z = [X; Y] (64 rows x 12288) in fp8-e4m3 (host casts;
quantization keeps worst rel err ~5e-4, well under the 2e-2 gate).
Every distance comes from the 64x64 Gram matrix G = Z Z^T:
    ||z_a - z_b||^2 = G[a,a] + G[b,b] - 2 G[a,b]
G is accumulated as 48 fp8 matmuls over PAIRS of 128-wide feature
chunks: the stationary is [128 feat, 128 cols] covering two chunks,
and the two diagonal 64x64 quadrants of the [128,128] PSUM tile hold
the two chunks' Gram contributions; off-diagonal quadrants are
cross-chunk garbage that is never read.  (One matmul per pair halves
the LDWEIGHTS+dispatch overhead vs per-chunk matmuls: 107ns vs 128ns
per two chunks, measured.)  The quadrants are never folded into one G:
the masked-reduce extraction runs on both quadrants (stacked on
partitions 0:64 / 64:128), and the tiny fp16 selection matmuls that
build the distance args contract over all 128 partitions with a
stacked identity, summing the two quadrants' contributions for free.

Input DMA is 6 non-uniform chunks on the Sync HWDGE ring, sized so
each chunk's arrival (issue + latency + wire + completion semaphore)
lands just before the 107ns/pair Gram consumption needs it; the
constants follow on the same ring so they never compete with input
chunks at the SDMA engines.  One ScalarE exp (scale = -beta/D,
per-partition bias = -beta/D * ||x_p||^2) evaluates all 9 distance
columns; the per-sample matmul + a column reduce finish the means.
"""

from contextlib import ExitStack

import numpy as np

import concourse.mybir as mybir
import concourse.tile as tile
from concourse import bacc
from concourse.bass_utils import run_bass_kernel_spmd

# problem shape (hardcoded per spec)
N, M, D = 32, 8, 12288
NUM_TIMESTEPS = 1000
BETA_START, BETA_END = 2.0, 0.1
LAMBDA_VAL = 1.0

NCORES = 8
NS = N // NCORES          # 4 samples per core
R = 2 * NS * M            # 64 Z-rows per core (32 x-rows then 32 y-rows)
NCH = D // 128            # 96 contraction chunks

FREE = NCH * R            # 6144 free columns of Z^T
NPAIR = NCH // 2          # 48 chunk pairs (one matmul each)
# Non-uniform input DMA chunks (columns of zt): sized so each chunk's
# arrival (issue + ~0.55us DMA latency + wire time + ~0.45us completion
# semaphore) lands just before the Gram matmuls (107ns/pair) need it.
CHUNK_COLS = [896, 896, 1280, 1536, 1024, 512]
CHUNK_PAIRS = [c // 128 for c in CHUNK_COLS]
assert sum(CHUNK_COLS) == FREE

# sel (f32) column layout: AAh(-0.5) | I4 | I4h(-0.5) | P4 | J8full
_A0, _I0, _H0, _P0, _J0 = 0, 32, 64, 96, 100
SELW = 108
# msk (bf16) column layout: BLK[32] | XY[32] | J8[8]
_B0, _X0, _JM = 0, 32, 64
MSKW = 72

IN_MODE = "fp8"           # "fp8" or "bf16" input/matmul dtype

F32 = mybir.dt.float32
F16 = mybir.dt.float16
BF16 = mybir.dt.bfloat16
FP8 = mybir.dt.float8e4


def _build_consts():
    p = np.arange(128)[:, None]
    i32 = np.arange(32)[None, :]
    blk = np.where(p < 64, p, p - 64)  # row index within quadrant
    inq = (p < 32) | ((p >= 64) & (p < 96))  # x-rows of either quadrant
    # AAh: sample-block selector (x-rows, both quadrants) scaled -0.5 to
    # undo the -2 baked into the block mask (rhsj carries -2*||x||^2)
    AA = (((blk // 8) == (i32 // 8)) & inq) * -0.5
    # I4: identity on each 32-row block -> matmul against a [128,1]
    # column sums the four blocks' entries (quadrant fold for free)
    I4 = ((p % 32) == i32) * 1.0
    I4h = I4 * -0.5
    P4 = ((p // 8) == np.arange(4)[None, :]) & (p < 32)
    J8 = ((blk % 8) == np.arange(8)[None, :]) & inq
    sel = np.concatenate([AA, I4, I4h, P4, J8], axis=1).astype(np.float16)

    # masks per quadrant row layout (rows 0:64 = quadrant 0, stacked):
    # BLK (cols 0:32): x-rows get -2 on their sample's 8 cols (diag
    #   included -> cm2[p, j(p)] = -2||x_p||^2 doubles as the norm source)
    # XY (cols 32:64): x-row p gets -2 at col p (the <x,y> cell of the
    #   quadrant's off-diag block); y-row q gets +1 at col q (||y||^2)
    q = np.arange(64)[:, None]
    BLKq = np.where(
        (q < 32) & ((q // 8) == (np.arange(32)[None, :] // 8)), -2.0, 0.0
    )
    XYq = np.zeros((64, 32), np.float32)
    for pp in range(32):
        XYq[pp, pp] = -2.0
        XYq[32 + pp, pp] = 1.0
    J8q = ((q % 8) == np.arange(8)[None, :]) & (q < 32)
    top = np.concatenate([BLKq, XYq, J8q], axis=1)  # [64, 72]
    msk = np.concatenate([top, top], axis=0)        # [128, 72] both quadrants
    import ml_dtypes

    return sel, msk.astype(ml_dtypes.bfloat16)


def _build_program(in_mode=IN_MODE):
    nc = bacc.Bacc("TRN2", target_bir_lowering=False)
    zdt = FP8 if in_mode == "fp8" else BF16
    zt = nc.dram_tensor("zt", [128, FREE], zdt, kind="ExternalInput")
    tq = nc.dram_tensor("tq", [N, 1], mybir.dt.int32, kind="ExternalInput")
    sel_d = nc.dram_tensor("sel", [128, SELW], F16, kind="ExternalInput")
    msk_d = nc.dram_tensor("msk", [128, MSKW], BF16, kind="ExternalInput")
    res_d = nc.dram_tensor("res", [NS, 4], F32, kind="ExternalOutput")

    add, mult, sub = (
        mybir.AluOpType.add,
        mybir.AluOpType.mult,
        mybir.AluOpType.subtract,
    )
    EXP = mybir.ActivationFunctionType.Exp

    with ExitStack() as ctx:
        tc = ctx.enter_context(tile.TileContext(nc))
        small = ctx.enter_context(tc.tile_pool(name="small", bufs=1))
        zin_p = ctx.enter_context(tc.tile_pool(name="zin", bufs=1))
        psum = ctx.enter_context(tc.tile_pool(name="psum", bufs=1, space="PSUM"))

        # --- input DMA first, on the Sync HWDGE ring ----------------------
        zc = []
        off = 0
        for i, cols in enumerate(CHUNK_COLS):
            z = zin_p.tile([128, cols], zdt, tag=f"zc{i}")
            nc.sync.dma_start(out=z, in_=zt[:, off : off + cols])
            zc.append(z)
            off += cols

        # tq rides the Scalar HWDGE ring (tiny, needed early for beta).
        # sel/msk aren't needed until extraction (~15us): issue them on
        # the Sync ring AFTER all zt chunks (FIFO per ring), so the big
        # const transfers never compete with input chunks at the SDMA
        # engines.
        tq_sb = small.tile([N, 1], mybir.dt.int32, tag="tq")
        nc.scalar.dma_start(out=tq_sb, in_=tq[:])
        sel = small.tile([128, SELW], F16, tag="sel")
        nc.sync.dma_start(out=sel, in_=sel_d[:])
        msk = small.tile([128, MSKW], BF16, tag="msk")
        nc.sync.dma_start(out=msk, in_=msk_d[:])

        # preload the Exp LUT while DMAs run
        warm = small.tile([1, 1], F32, tag="warm")
        nc.vector.memset(warm, 0.0)
        nc.scalar.activation(out=warm, in_=warm, func=EXP)

        npair = float(M * (M - 1))

        # beta pipeline: bvec[p] = -beta[s(p)]/D on partitions 0:32
        tf = small.tile([N, 1], F32, tag="tf")
        nc.vector.tensor_copy(out=tf, in_=tq_sb)  # int32 -> f32
        bvec = small.tile([N, 1], F32, tag="bvec")
        nc.vector.tensor_scalar(
            out=bvec,
            in0=tf,
            scalar1=(BETA_START - BETA_END) / ((NUM_TIMESTEPS - 1) * D),
            scalar2=-BETA_START / D,
            op0=mult,
            op1=add,
        )
        # --- Gram: P[128,128] += S^T S over 48 chunk pairs ----------------
        P = psum.tile([128, 128], F32, tag="P")
        k = 0
        for c, npr in enumerate(CHUNK_PAIRS):
            for j in range(npr):
                sl = zc[c][:, j * 128 : (j + 1) * 128]
                nc.tensor.matmul(
                    P, lhsT=sl, rhs=sl, start=(k == 0), stop=(k == NPAIR - 1)
                )
                k += 1

        # --- extraction on both quadrants (stacked on partitions) ---------
        # One masked multiply per quadrant covers everything: cols 0:32
        # hold the -2*G sample blocks (diag included), cols 32:64 hold
        # -2<x,y> (x-rows) / ||y||^2 (y-rows).
        m1big = small.tile([128, 64], F32, tag="m1big")
        nc.vector.tensor_tensor(
            out=m1big[0:64, :], in0=P[0:64, 0:64],
            in1=msk[0:64, 0:64], op=mult,
        )
        nc.vector.tensor_tensor(
            out=m1big[64:128, :], in0=P[64:128, 64:128],
            in1=msk[64:128, 0:64], op=mult,
        )
        # cm2[p, f] = -2 <x_p, x_{s(p)*8+f}> (per quadrant)
        cm2 = small.tile([128, 8], F16, tag="cm2")
        with nc.allow_low_precision("fp16 holds ~5e4 Gram entries at 5e-4 rel"):
            nc.vector.reduce_sum(
                out=cm2,
                in_=m1big[:, 0:32].rearrange("p (g f) -> p f g", g=NS),
                axis=mybir.AxisListType.X,
            )
        # One STT gives both: its main out cm2*J8 IS rhsj (the -2||x||^2
        # value at col j(p), zero elsewhere), and its accumulator gives
        # xnq[p] = cm2[p, j(p)] = -2 ||x_p||^2 for the exp bias.
        rhsj = small.tile([128, 8], F16, tag="rhsj")
        xnq = small.tile([128, 1], F16, tag="xnq")
        with nc.allow_low_precision("fp16 holds ~2.5e4 norms at 5e-4 rel"):
            nc.vector.scalar_tensor_tensor(
                out=rhsj, in0=cm2, scalar=1.0, in1=msk[:, _JM : _JM + 8],
                op0=mult, op1=mult, accum_out=xnq,
            )
        # r1xy[p] = -2<x,y> (x-rows) / ||y||^2 (y-rows), per quadrant
        r1xy = small.tile([128, 1], F16, tag="r1xy")
        with nc.allow_low_precision("fp16 holds ~2.5e4 norms at 5e-4 rel"):
            nc.vector.reduce_sum(
                out=r1xy, in_=m1big[:, 32:64], axis=mybir.AxisListType.X
            )

        # --- selection matmuls (each also folds the two quadrants) --------
        # xnp[p] = ||x_p||^2 total;  P9[:,8] = ||y_p||^2 - 2<x_p,y_p>;
        # P9[:,0:8] = ||x_{s,f}||^2 - 2<x_p, x_{s,f}>
        xnp = psum.tile([32, 1], F32, tag="xnp")
        nc.tensor.matmul(
            xnp, lhsT=sel[:, _H0 : _H0 + 32], rhs=xnq,
            start=True, stop=True,
        )
        P9 = psum.tile([32, 9], F32, tag="P9")
        nc.tensor.matmul(
            P9[:, 8:9], lhsT=sel[:, _I0 : _I0 + 32], rhs=r1xy,
            start=True, stop=True,
        )
        nc.tensor.matmul(
            P9[:, 0:8], lhsT=sel[:, _I0 : _I0 + 32], rhs=cm2,
            start=True, stop=False,
        )
        nc.tensor.matmul(
            P9[:, 0:8], lhsT=sel[:, _A0 : _A0 + 32], rhs=rhsj,
            start=False, stop=True,
        )

        # bias = -beta/D * ||x_p||^2 folds the per-row norm into the exp
        bxn = small.tile([32, 1], F32, tag="bxn")
        nc.vector.tensor_tensor(out=bxn, in0=bvec, in1=xnp, op=mult)

        # e9 = exp(-beta/D * (d2 terms)); cols 0:8 pair args, col 8 xy arg
        e9 = small.tile([32, 9], F16, tag="e9")
        nc.scalar.activation(
            out=e9, in_=P9, func=EXP, scale=bvec, bias=bxn
        )

        # per-sample sums over the 8 population rows
        psm9 = psum.tile([NS, 9], F32, tag="psm9")
        nc.tensor.matmul(
            psm9, lhsT=sel[0:32, _P0 : _P0 + NS], rhs=e9, start=True, stop=True
        )

        # finals: [score, conf, inter, inter_mult]
        pr = small.tile([NS, 1], F32, tag="pr")
        nc.vector.reduce_sum(
            out=pr, in_=psm9[:, 0:8], axis=mybir.AxisListType.X
        )
        fin = small.tile([NS, 4], F32, tag="fin")
        nc.vector.tensor_scalar(
            out=fin[:, 1:2], in0=psm9[:, 8:9], scalar1=1.0 / M, scalar2=None,
            op0=mult,
        )
        nc.vector.tensor_scalar(
            out=fin[:, 2:3], in0=pr,
            scalar1=1.0 / npair, scalar2=-M / npair, op0=mult, op1=add,
        )
        half_lam = LAMBDA_VAL / 2.0
        nc.vector.tensor_scalar(
            out=fin[:, 3:4], in0=pr,
            scalar1=half_lam / npair, scalar2=-M * half_lam / npair,
            op0=mult, op1=add,
        )
        nc.vector.tensor_tensor(
            out=fin[:, 0:1], in0=fin[:, 3:4], in1=fin[:, 1:2], op=sub
        )
        nc.scalar.dma_start(out=res_d[:], in_=fin)

    nc.compile()
    return nc


_PROG = {}
_CONSTS = None


def _get_prog(in_mode=IN_MODE):
    if in_mode not in _PROG:
        _PROG[in_mode] = _build_program(in_mode)
    return _PROG[in_mode]


def _make_in_maps(x, y, t, in_mode=IN_MODE):
    global _CONSTS
    if _CONSTS is None:
        _CONSTS = _build_consts()
    sel, msk = _CONSTS
    import ml_dtypes

    zdt = ml_dtypes.float8_e4m3 if in_mode == "fp8" else ml_dtypes.bfloat16
    in_maps = []
    for c in range(NCORES):
        xc = x[c * NS : (c + 1) * NS].reshape(NS * M, D)
        yc = y[c * NS : (c + 1) * NS].reshape(NS * M, D)
        z = np.concatenate([xc, yc], axis=0)  # [64, D]
        # feature-major: zt[p, k*64 + r] = z[r, k*128 + p]
        ztc = np.ascontiguousarray(
            z.reshape(R, NCH, 128).transpose(2, 1, 0).reshape(128, FREE),
            dtype=zdt,
        )
        trep = np.repeat(t[c * NS : (c + 1) * NS], M).reshape(N, 1)
        in_maps.append(
            {
                "zt": ztc,
                "tq": np.ascontiguousarray(trep, dtype=np.int32),
                "sel": sel,
                "msk": msk,
            }
        )
    return in_maps


def _run(x, y, t, trace=False, in_mode=IN_MODE, **spmd_kwargs):
    x = np.asarray(x, dtype=np.float32)
    y = np.asarray(y, dtype=np.float32)
    t = np.asarray(t, dtype=np.int32)
    nc = _get_prog(in_mode)
    in_maps = _make_in_maps(x, y, t, in_mode)
    br = run_bass_kernel_spmd(
        nc, in_maps, list(range(NCORES)), trace=trace, **spmd_kwargs
    )
    out = np.concatenate(
        [np.asarray(r["res"], dtype=np.float32) for r in br.results], axis=0
    )  # [32, 4]
    outs = tuple(np.ascontiguousarray(out[:, i]) for i in range(4))
    return outs, br


def kernel(x, y, t):
    """(score, confinement, interaction, interaction_mult), each [32] f32."""
    outs, _ = _run(x, y, t)
    return outs
